# revision 18
# baseline (speedup 1.0000x reference)
"""Fused attention+MoE block on 8 trn2 NeuronCores.

Sharding: tensor-parallel attention (4 q-heads + 1 KV-head per core),
on-device chunked AllReduce of attention partials, expert-parallel MoE
(1 expert per core). Activations live in a transposed [feature, token]
layout so every matmul is weight-stationary with no activation
transposes. The host pre-transposes/chunks inputs, folds the (unit) ln
weights into adjacent matmuls, and sums the 8 per-core partial outputs.
"""

import numpy as np
import ml_dtypes

import concourse.bass as bass
import concourse.mybir as mybir
import concourse.tile as tile
from concourse.bass_utils import run_bass_kernel_spmd
from concourse.masks import make_identity
from concourse.vector_clock import ScopedClock

F32 = mybir.dt.float32
F32R = mybir.dt.float32r
F16 = mybir.dt.float16
BF16 = mybir.dt.bfloat16
U32 = mybir.dt.uint32
AF = mybir.ActivationFunctionType
OP = mybir.AluOpType

B, T, C = 2, 1024, 2048
H, KV, D = 32, 8, 128
E, F, TOPK = 8, 768, 2
N = B * T
CK = C // 128         # 16
NBLK = 4              # token blocks of 512
TB = 512
HQ = H // E           # 4
FK = F // 128         # 6
FT = 2 * F // 128     # 12
EPS = 1e-6
N_CORES = 8

# ---------------------------------------------------------------------------
# walrus here rejects >1 sync-wait per instruction; split extras onto NoOps.


class _SplitDrainTileContext(tile.TileContext):
    def _drain_and_barrier(self, tick_clock, wait_clock):
        drain_inst = self.nc.sync.drain()
        wait_clock.add_sem_waits(
            drain_inst.ins, ScopedClock({None: tick_clock.global_clock})
        )
        si = drain_inst.ins.sync_info
        if si is not None and len(si.on_wait) > 1:
            ow = list(si.on_wait)
            drain_inst.ins.sync_info = mybir.SyncInfo(
                on_wait=ow[:1], on_update=list(si.on_update)
            )
            rest = ow[1:]
            while rest:
                extra = self.nc.sync.drain()
                extra.ins.sync_info = mybir.SyncInfo(on_wait=rest[:1], on_update=[])
                rest = rest[1:]
        self.nc.all_engine_barrier()
        assert self.sems is not None
        popped = self.nc._tile_sem_poison_stack.pop()
        assert popped is self._sem_poison
        self.nc.clear_and_free_semaphores(list(self.sems.allocated().values()))
        self.nc.all_engine_barrier()


def _split_multi_waits(nc):
    for bb in nc.main_func.blocks:
        insts = list(bb.instructions)
        out = []
        changed = False
        for ins in insts:
            si = ins.sync_info
            if si is not None and len(si.on_wait) > 1:
                ow = list(si.on_wait)
                for w in ow[:-1]:
                    nop = mybir.InstNoOp(name=f"waitnop-{nc.next_id()}", ins=[], outs=[])
                    nop.engine = ins.engine
                    nop.sync_info = mybir.SyncInfo(on_wait=[w], on_update=[])
                    out.append(nop)
                ins.sync_info = mybir.SyncInfo(
                    on_wait=[ow[-1]], on_update=list(si.on_update)
                )
                changed = True
            out.append(ins)
        if changed:
            bb.instructions = out


# ---------------------------------------------------------------------------


def build_nc(phases='ABC'):
    nc = bass.Bass("TRN2", target_bir_lowering=False, debug=False, num_devices=N_CORES)

    xT = nc.dram_tensor("xT", [128, CK, N], F32, kind="ExternalInput")
    qw = nc.dram_tensor("qw", [128, CK, HQ * 128], BF16, kind="ExternalInput")
    kw = nc.dram_tensor("kw", [128, CK, 128], BF16, kind="ExternalInput")
    vw = nc.dram_tensor("vw", [128, CK, 128], BF16, kind="ExternalInput")
    ow = nc.dram_tensor("ow", [128, CK, HQ, 128], BF16, kind="ExternalInput")
    gatew = nc.dram_tensor("gatew", [128, CK, E], F32, kind="ExternalInput")
    guw = nc.dram_tensor("guw", [128, FT, CK, 128], BF16, kind="ExternalInput")
    dww = nc.dram_tensor("dww", [128, CK, FK, 128], BF16, kind="ExternalInput")
    cosT = nc.dram_tensor("cosT", [128, T], F32, kind="ExternalInput")
    sinTs = nc.dram_tensor("sinTs", [128, T], F32, kind="ExternalInput")
    masks = nc.dram_tensor("masks", [128, 4, TB], BF16, kind="ExternalInput")
    rstd1 = nc.dram_tensor("rstd1", [1, N], F32, kind="ExternalInput")
    qnw = nc.dram_tensor("qnw", [128, 1], F32, kind="ExternalInput")
    knw = nc.dram_tensor("knw", [128, 1], F32, kind="ExternalInput")
    expid = nc.dram_tensor("expid", [128, 1], F32, kind="ExternalInput")
    prot = nc.dram_tensor("prot", [128, 128], F32, kind="ExternalInput")

    y = nc.dram_tensor("y", [128, CK, N], F32, kind="ExternalOutput")

    with _SplitDrainTileContext(nc) as tc:
        with (
            tc.tile_pool(name="const", bufs=1) as cpool,
            tc.tile_pool(name="dram", bufs=1, space="DRAM") as dram,
            tc.tile_pool(name="ps", bufs=1, space="PSUM") as ps,
            tc.tile_pool(name="big", bufs=1) as big,
            tc.tile_pool(name="work", bufs=3) as wk,
        ):
            MM = dict(tag="mm", bufs=6)
            ROW = dict(tag="row", bufs=2)
            TF = dict(tag="t512f", bufs=4)    # f32 [128,TB] transients
            TBF = dict(tag="t512b", bufs=4)   # bf16 [128,TB] transients
            TR = dict(tag="t512r", bufs=2)    # f32r [128,TB] transients
            IO = dict(tag="io512", bufs=3)    # f32 [128,TB] DMA-facing
            RWF = dict(tag="rowf", bufs=2)    # f32 [1,TB]
            RWR = dict(tag="rowr", bufs=2)    # f32r [1,TB]
            TINY = dict(tag="tiny", bufs=16)  # [128,<=8] f32/u32

            # ---- constants ----
            ident = cpool.tile([128, 128], BF16)
            make_identity(nc, ident)
            identf = cpool.tile([128, 128], F32)
            make_identity(nc, identf)
            ones_bf = cpool.tile([128, 1], BF16)
            nc.vector.memset(ones_bf, 1.0)
            onesrow_f = cpool.tile([1, 128], F32)
            nc.vector.memset(onesrow_f, 1.0)
            onesrow = cpool.tile([1, 128], F32R)
            nc.vector.tensor_copy(out=onesrow, in_=onesrow_f)
            ones_col_f = cpool.tile([128, 1], F32)
            nc.vector.memset(ones_col_f, 1.0)
            ones_r = cpool.tile([128, 1], F32R)
            nc.vector.tensor_copy(out=ones_r, in_=ones_col_f)
            prot_f = cpool.tile([128, 128], F32)
            nc.sync.dma_start(out=prot_f, in_=prot[:])
            prot_r = cpool.tile([128, 128], F32R)
            nc.vector.tensor_copy(out=prot_r, in_=prot_f)
            bias_q = cpool.tile([1, 1], F32)
            nc.vector.memset(bias_q, float(D) * EPS)
            bias_eps = cpool.tile([1, 1], F32)
            nc.vector.memset(bias_eps, EPS)

            cosT_sb = cpool.tile([128, T], F32)
            nc.sync.dma_start(out=cosT_sb, in_=cosT[:])
            sinTs_sb = cpool.tile([128, T], F32)
            nc.sync.dma_start(out=sinTs_sb, in_=sinTs[:])
            masks_sb = cpool.tile([128, 4, TB], BF16)
            nc.sync.dma_start(out=masks_sb, in_=masks[:])
            qnw_sb = cpool.tile([128, 1], F32)
            nc.sync.dma_start(out=qnw_sb, in_=qnw[:])
            knw_sb = cpool.tile([128, 1], F32)
            nc.sync.dma_start(out=knw_sb, in_=knw[:])
            expid_sb = cpool.tile([128, 1], F32)
            nc.sync.dma_start(out=expid_sb, in_=expid[:])
            gatew_sb = cpool.tile([128, CK, E], F32)
            nc.sync.dma_start(out=gatew_sb, in_=gatew[:])
            gatew_r = cpool.tile([128, CK, E], F32R)
            nc.vector.tensor_copy(out=gatew_r, in_=gatew_sb)

            qw_sb = big.tile([128, CK, HQ * 128], BF16)
            nc.sync.dma_start(out=qw_sb, in_=qw[:])
            kw_sb = big.tile([128, CK, 128], BF16)
            nc.sync.dma_start(out=kw_sb, in_=kw[:])
            vw_sb = big.tile([128, CK, 128], BF16)
            nc.sync.dma_start(out=vw_sb, in_=vw[:])

            qT_sb = big.tile([128, HQ, N], BF16)           # 2MB
            kT_sb = big.tile([128, N], BF16)               # 0.5MB
            vnat_sb = big.tile([128, N // 128, 128], BF16)  # 0.5MB
            xn_bf = big.tile([128, CK, TB], BF16)           # 2MB

            ar_in = [dram.tile([128, CK, TB], F16, name=f"arin{g}") for g in range(NBLK)]
            ar_out = [dram.tile([128, CK, TB], F16, name=f"arout{g}") for g in range(NBLK)]

            def bcast(row_f32_ap):
                """[1,TB] f32 -> SBUF [128,TB] f32 via K=1 f32r matmul."""
                rr = wk.tile([1, TB], F32R, **RWR)
                nc.vector.tensor_copy(out=rr, in_=row_f32_ap)
                bc_ps = ps.tile([128, TB], F32, **MM)
                nc.tensor.matmul(bc_ps, onesrow, rr, start=True, stop=True)
                bc = wk.tile([128, TB], F32, tag="bcs", bufs=4)
                nc.vector.tensor_copy(out=bc, in_=bc_ps)
                return bc

            def colsum_rstd(feat_ps, scale, bias_ap):
                """rsqrt(scale*colsum(feat^2)+bias) -> [1,TB] f32 row."""
                sq = wk.tile([128, TB], BF16, **TBF)
                nc.scalar.activation(out=sq, in_=feat_ps, func=AF.Square)
                ssum = ps.tile([1, TB], F32, **ROW)
                nc.tensor.matmul(ssum, ones_bf, sq, start=True, stop=True)
                srow = wk.tile([1, TB], F32, **RWF)
                nc.scalar.activation(out=srow, in_=ssum, func=AF.Sqrt,
                                     scale=scale, bias=bias_ap)
                rrow = wk.tile([1, TB], F32, **RWF)
                nc.vector.reciprocal(out=rrow, in_=srow)
                return rrow

            def rope_norm(feat_ps, g, w_sb, rstd_bc, out_ap):
                """out = rope(w * feat * rstd); out_ap bf16 [128,TB].
                rotate_half is a signed partition permutation -> PE matmul."""
                tcol = (g % 2) * TB
                qhat = wk.tile([128, TB], F32, **TF)
                nc.vector.scalar_tensor_tensor(
                    out=qhat, in0=feat_ps, scalar=w_sb, in1=rstd_bc,
                    op0=OP.mult, op1=OP.mult,
                )
                qhat_r = wk.tile([128, TB], F32R, **TR)
                nc.vector.tensor_copy(out=qhat_r, in_=qhat)
                rot_ps = ps.tile([128, TB], F32, **MM)
                nc.tensor.matmul(rot_ps, prot_r, qhat_r, start=True, stop=True)
                qc = wk.tile([128, TB], F32, **TF)
                nc.vector.tensor_tensor(
                    out=qc, in0=qhat, in1=cosT_sb[:, tcol:tcol + TB], op=OP.mult,
                )
                rsm = wk.tile([128, TB], F32, **TF)
                nc.vector.tensor_tensor(
                    out=rsm, in0=rot_ps, in1=sinTs_sb[:, tcol:tcol + TB], op=OP.mult,
                )
                nc.vector.tensor_tensor(out=out_ap, in0=qc, in1=rsm, op=OP.add)

            # =========================== Phase A: QKV ======================
            def phaseA(g):
                n0 = g * TB
                xtb = wk.tile([128, CK, TB], BF16, tag="xtb", bufs=1)
                for kc in range(CK):
                    xch = wk.tile([128, TB], F32, **IO)
                    nc.sync.dma_start(out=xch, in_=xT[:, kc, n0:n0 + TB])
                    nc.gpsimd.tensor_copy(out=xtb[:, kc, :], in_=xch)
                r1row = wk.tile([1, TB], F32, **RWF)
                nc.sync.dma_start(out=r1row, in_=rstd1[:, n0:n0 + TB])
                r1bc = bcast(r1row)

                for hd in range(HQ):
                    q_ps = ps.tile([128, TB], F32, **MM)
                    for kc in range(CK):
                        nc.tensor.matmul(
                            q_ps, qw_sb[:, kc, hd * 128:(hd + 1) * 128],
                            xtb[:, kc, :], start=(kc == 0), stop=(kc == CK - 1),
                        )
                    # rstd with the 1/sqrt(D) score scale folded in:
                    # 1/sqrt(D*(mean+eps)) = 1/sqrt(colsum + D*eps)
                    qrow = colsum_rstd(q_ps, 1.0, bias_q)
                    qbc = bcast(qrow)
                    rope_norm(q_ps, g, qnw_sb, qbc, qT_sb[:, hd, n0:n0 + TB])

                k_ps = ps.tile([128, TB], F32, **MM)
                for kc in range(CK):
                    nc.tensor.matmul(k_ps, kw_sb[:, kc, :], xtb[:, kc, :],
                                     start=(kc == 0), stop=(kc == CK - 1))
                krow = colsum_rstd(k_ps, 1.0 / D, bias_eps)
                kbc = bcast(krow)
                rope_norm(k_ps, g, knw_sb, kbc, kT_sb[:, n0:n0 + TB])

                v_ps = ps.tile([128, TB], F32, **MM)
                for kc in range(CK):
                    nc.tensor.matmul(v_ps, vw_sb[:, kc, :], xtb[:, kc, :],
                                     start=(kc == 0), stop=(kc == CK - 1))
                vhat = wk.tile([128, TB], BF16, **TBF)
                nc.vector.tensor_tensor(out=vhat, in0=v_ps, in1=r1bc, op=OP.mult)
                for cc in range(TB // 128):
                    vtr = ps.tile([128, 128], BF16, **MM)
                    nc.tensor.transpose(vtr, vhat[:, cc * 128:(cc + 1) * 128], ident)
                    nc.vector.tensor_copy(out=vnat_sb[:, g * 4 + cc, :], in_=vtr)

            # ====================== Phase B: attention =====================
            def phaseB(g):
                b, j = divmod(g, 2)
                q0 = b * T + j * TB
                avT = wk.tile([128, HQ, TB], BF16, tag="avT", bufs=2)
                for hd in range(HQ):
                    ntk = 4 * j + 4
                    av_ps = ps.tile([128, TB], F32, **MM)
                    den_ps = ps.tile([1, TB], F32, **ROW)
                    for i in range(ntk):
                        tk0 = b * T + i * 128
                        sc_ps = ps.tile([128, TB], F32, **MM)
                        nc.tensor.matmul(sc_ps, kT_sb[:, tk0:tk0 + 128],
                                         qT_sb[:, hd, q0:q0 + TB], start=True, stop=True)
                        ex = wk.tile([128, TB], BF16, **TBF)
                        s = i - 4 * j
                        if s < 0:
                            nc.scalar.activation(out=ex, in_=sc_ps, func=AF.Exp)
                        else:
                            ext = wk.tile([128, TB], BF16, **TBF)
                            nc.scalar.activation(out=ext, in_=sc_ps, func=AF.Exp)
                            nc.vector.tensor_tensor(out=ex, in0=ext,
                                                    in1=masks_sb[:, s, :], op=OP.mult)
                        nc.tensor.matmul(den_ps, ones_bf, ex,
                                         start=(i == 0), stop=(i == ntk - 1))
                        nc.tensor.matmul(av_ps, vnat_sb[:, b * 8 + i, :], ex,
                                         start=(i == 0), stop=(i == ntk - 1))
                    dsb = wk.tile([1, TB], F32, **RWF)
                    nc.vector.tensor_copy(out=dsb, in_=den_ps)
                    rec = wk.tile([1, TB], F32, **RWF)
                    nc.vector.reciprocal(out=rec, in_=dsb)
                    rbc = bcast(rec)
                    nc.vector.tensor_tensor(out=avT[:, hd, :], in0=av_ps, in1=rbc,
                                            op=OP.mult)
                for m in range(CK):
                    owt = wk.tile([128, HQ, 128], BF16, tag="owt", bufs=2)
                    nc.sync.dma_start(out=owt, in_=ow[:, m, :, :])
                    att_ps = ps.tile([128, TB], F32, **MM)
                    for hk in range(HQ):
                        nc.tensor.matmul(att_ps, owt[:, hk, :],
                                         avT[:, hk, :], start=(hk == 0), stop=(hk == HQ - 1))
                    att_sb = wk.tile([128, TB], F16, tag="t512h", bufs=4)
                    nc.scalar.copy(out=att_sb, in_=att_ps)
                    nc.sync.dma_start(out=ar_in[g][:, m, :], in_=att_sb)
                nc.gpsimd.collective_compute(
                    "AllReduce", OP.add,
                    replica_groups=[list(range(N_CORES))],
                    ins=[ar_in[g].opt()], outs=[ar_out[g].opt()],
                )

            # ========================= Phase C: MoE ========================
            def phaseC(g):
                n0 = g * TB
                lg_ps = ps.tile([E, TB], F32, **ROW)
                den2_ps = ps.tile([1, TB], F32, **ROW)
                for kc in range(CK):
                    arch = wk.tile([128, TB], F16, tag="t512h", bufs=4)
                    nc.sync.dma_start(out=arch, in_=ar_out[g][:, kc, :])
                    xch2 = wk.tile([128, TB], F32, **IO)
                    nc.sync.dma_start(out=xch2, in_=xT[:, kc, n0:n0 + TB])
                    xn_f = wk.tile([128, TB], F32, **TF)
                    nc.vector.tensor_tensor(out=xn_f, in0=arch, in1=xch2, op=OP.add)
                    xn = wk.tile([128, TB], F32R, **TR)
                    nc.vector.tensor_copy(out=xn, in_=xn_f)
                    nc.tensor.matmul(lg_ps, gatew_r[:, kc, :], xn,
                                     start=(kc == 0), stop=(kc == CK - 1))
                    xn2 = wk.tile([128, TB], BF16, **TBF)
                    nc.scalar.activation(out=xn2, in_=xn_f, func=AF.Square)
                    nc.tensor.matmul(den2_ps, ones_bf, xn2,
                                     start=(kc == 0), stop=(kc == CK - 1))
                    nc.gpsimd.tensor_copy(out=xn_bf[:, kc, :], in_=xn_f)
                s2 = wk.tile([1, TB], F32, **RWF)
                nc.scalar.activation(out=s2, in_=den2_ps, func=AF.Sqrt,
                                     scale=1.0 / C, bias=bias_eps)
                rstd2 = wk.tile([1, TB], F32, tag="rstd2", bufs=1)
                nc.vector.reciprocal(out=rstd2, in_=s2)
                lg_sb = wk.tile([E, TB], F32, tag="lgsb", bufs=1)
                nc.vector.tensor_copy(out=lg_sb, in_=lg_ps)

                combrow = wk.tile([1, TB], F32, tag="combrow", bufs=1)
                for t in range(TB // 128):
                    c0 = t * 128
                    r2T_ps = ps.tile([128, 1], F32, **MM)
                    nc.tensor.transpose(r2T_ps, rstd2[:, c0:c0 + 128],
                                        identf[0:1, 0:1])
                    r2T = wk.tile([128, 1], F32, **TINY)
                    nc.vector.tensor_copy(out=r2T, in_=r2T_ps)
                    ln_ps = ps.tile([128, E], F32, **MM)
                    nc.tensor.transpose(ln_ps, lg_sb[:, c0:c0 + 128],
                                        identf[0:E, 0:E])
                    lsc = wk.tile([128, E], F32, **TINY)
                    nc.vector.tensor_scalar(out=lsc, in0=ln_ps, scalar1=r2T,
                                            scalar2=None, op0=OP.mult)
                    vals = wk.tile([128, 8], F32, **TINY)
                    idxs = wk.tile([128, 8], U32, **TINY)
                    nc.vector.max_with_indices(vals, idxs, lsc)
                    dlt = wk.tile([128, 1], F32, **TINY)
                    nc.vector.tensor_tensor(out=dlt, in0=vals[:, 1:2],
                                            in1=vals[:, 0:1], op=OP.subtract)
                    w1 = wk.tile([128, 1], F32, **TINY)
                    nc.scalar.activation(out=w1, in_=dlt, func=AF.Sigmoid,
                                         scale=-1.0)
                    w2 = wk.tile([128, 1], F32, **TINY)
                    nc.vector.tensor_scalar(out=w2, in0=w1, scalar1=-1.0,
                                            scalar2=1.0, op0=OP.mult, op1=OP.add)
                    idxf = wk.tile([128, 2], F32, **TINY)
                    nc.vector.tensor_copy(out=idxf, in_=idxs[:, 0:2])
                    eq1 = wk.tile([128, 1], F32, **TINY)
                    nc.vector.tensor_tensor(out=eq1, in0=idxf[:, 0:1],
                                            in1=expid_sb, op=OP.is_equal)
                    eq2 = wk.tile([128, 1], F32, **TINY)
                    nc.vector.tensor_tensor(out=eq2, in0=idxf[:, 1:2],
                                            in1=expid_sb, op=OP.is_equal)
                    r1m = wk.tile([128, 1], F32, **TINY)
                    nc.vector.tensor_tensor(out=r1m, in0=eq1, in1=w1, op=OP.mult)
                    r2m = wk.tile([128, 1], F32, **TINY)
                    nc.vector.tensor_tensor(out=r2m, in0=eq2, in1=w2, op=OP.mult)
                    routed = wk.tile([128, 1], F32, **TINY)
                    nc.vector.tensor_tensor(out=routed, in0=r1m, in1=r2m, op=OP.add)
                    comb = wk.tile([128, 1], F32, **TINY)
                    nc.vector.tensor_tensor(out=comb, in0=routed, in1=r2T,
                                            op=OP.mult)
                    cT_ps = ps.tile([1, 128], F32, **MM)
                    nc.tensor.transpose(cT_ps, comb, identf)
                    nc.vector.tensor_copy(out=combrow[:, c0:c0 + 128], in_=cT_ps)

                r2bc = bcast(rstd2)
                cbc = bcast(combrow)

                prod = wk.tile([128, FK, TB], BF16, tag="prod", bufs=1)
                for fg in range(FK):
                    guw_u = wk.tile([128, CK, 128], BF16, tag="guwt", bufs=2)
                    nc.sync.dma_start(out=guw_u, in_=guw[:, FK + fg, :, :])
                    guw_g = wk.tile([128, CK, 128], BF16, tag="guwt", bufs=2)
                    nc.sync.dma_start(out=guw_g, in_=guw[:, fg, :, :])
                    u_ps = ps.tile([128, TB], F32, **MM)
                    for kc in range(CK):
                        nc.tensor.matmul(u_ps, guw_u[:, kc, :], xn_bf[:, kc, :],
                                         start=(kc == 0), stop=(kc == CK - 1))
                    g_ps = ps.tile([128, TB], F32, **MM)
                    for kc in range(CK):
                        nc.tensor.matmul(g_ps, guw_g[:, kc, :], xn_bf[:, kc, :],
                                         start=(kc == 0), stop=(kc == CK - 1))
                    gsc = wk.tile([128, TB], F32, **TF)
                    nc.vector.tensor_tensor(out=gsc, in0=g_ps, in1=r2bc,
                                            op=OP.mult)
                    sil = wk.tile([128, TB], F32, **TF)
                    nc.scalar.activation(out=sil, in_=gsc, func=AF.Silu)
                    ucm = wk.tile([128, TB], F32, **TF)
                    nc.vector.tensor_tensor(out=ucm, in0=u_ps, in1=cbc,
                                            op=OP.mult)
                    nc.vector.tensor_tensor(out=prod[:, fg, :], in0=sil,
                                            in1=ucm, op=OP.mult)
                for m in range(CK):
                    dwt = wk.tile([128, FK, 128], BF16, tag="dwt", bufs=2)
                    nc.sync.dma_start(out=dwt, in_=dww[:, m, :, :])
                    eo_ps = ps.tile([128, TB], F32, **MM)
                    for fk in range(FK):
                        nc.tensor.matmul(eo_ps, dwt[:, fk, :], prod[:, fk, :],
                                         start=(fk == 0), stop=(fk == FK - 1))
                    attb = wk.tile([128, TB], F16, tag="t512h", bufs=4)
                    nc.sync.dma_start(out=attb, in_=ar_in[g][:, m, :])
                    y_sb = wk.tile([128, TB], F32, **IO)
                    nc.vector.tensor_tensor(out=y_sb, in0=eo_ps, in1=attb,
                                            op=OP.add)
                    nc.sync.dma_start(out=y[:, m, n0:n0 + TB], in_=y_sb)

            # driver: interleave so AR(g) issues early and C(g-1) overlaps B
            for g in range(NBLK):
                phaseA(g)
                if 'B' in phases:
                    phaseB(g)
                if 'C' in phases and g >= 1:
                    phaseC(g - 1)
            if 'C' in phases:
                phaseC(NBLK - 1)

    _split_multi_waits(nc)
    return nc


# ---------------------------------------------------------------------------

_NC_CACHE = {}


def _get_nc():
    if "nc" not in _NC_CACHE:
        _NC_CACHE["nc"] = build_nc()
    return _NC_CACHE["nc"]


def _chunk_pm(a, nchunk):
    """[nchunk*128, free...] -> [128, nchunk, free...]"""
    return np.ascontiguousarray(
        a.reshape(nchunk, 128, *a.shape[1:]).transpose(1, 0, *range(2, a.ndim + 1))
    )


def prepare_in_maps(x, cos, sin, ln1_w, q_w, k_w, v_w, o_w, qn_w, kn_w, ln2_w,
                    gate_w, gate_up_w, down_w):
    bf = ml_dtypes.bfloat16
    x = np.asarray(x, dtype=np.float32)
    x_flat = x.reshape(N, C)

    xT = _chunk_pm(np.ascontiguousarray(x_flat.T), CK)
    rstd1 = (1.0 / np.sqrt((x_flat.astype(np.float64) ** 2).mean(axis=1) + EPS)
             ).astype(np.float32)[None, :]

    ln1 = np.asarray(ln1_w, dtype=np.float32)[:, None]
    ln2 = np.asarray(ln2_w, dtype=np.float32)[:, None]
    qwf = np.asarray(q_w, dtype=np.float32) * ln1
    kwf = np.asarray(k_w, dtype=np.float32) * ln1
    vwf = np.asarray(v_w, dtype=np.float32) * ln1
    gatewf = np.asarray(gate_w, dtype=np.float32) * ln2
    guwf = np.asarray(gate_up_w, dtype=np.float32) * ln2[None]
    dwf = np.asarray(down_w, dtype=np.float32)
    owf = np.asarray(o_w, dtype=np.float32)

    cos0 = np.asarray(cos, dtype=np.float32)[0]
    sin0 = np.asarray(sin, dtype=np.float32)[0]
    cosT = np.ascontiguousarray(cos0.T)
    sinT = np.ascontiguousarray(sin0.T)
    protm = np.zeros((128, 128), dtype=np.float32)
    for m in range(64):
        protm[m + 64, m] = -1.0
    for m in range(64, 128):
        protm[m - 64, m] = 1.0

    r = np.arange(128)[:, None]
    col = np.arange(TB)[None, :]
    masks = np.stack(
        [(col >= r + 128 * s).astype(bf) for s in range(4)], axis=1
    )

    in_maps = []
    for c in range(N_CORES):
        # guw host layout: [128, FT, CK, 128]
        gslice = guwf[c].astype(bf)                      # [C, 2F]
        g4 = gslice.reshape(CK, 128, FT, 128)            # kc, p, ft, d
        guw_h = np.ascontiguousarray(g4.transpose(1, 2, 0, 3))
        oslice = owf[512 * c:512 * (c + 1), :].astype(bf)  # [512, C]
        o4 = oslice.reshape(HQ, 128, CK, 128)            # hk, p, m, d
        ow_h = np.ascontiguousarray(o4.transpose(1, 2, 0, 3))
        dslice = dwf[c].astype(bf)                       # [F, C]
        d4 = dslice.reshape(FK, 128, CK, 128)            # fk, p, m, d
        dww_h = np.ascontiguousarray(d4.transpose(1, 2, 0, 3))
        in_maps.append({
            "xT": xT,
            "qw": _chunk_pm(qwf[:, 512 * c:512 * (c + 1)].astype(bf), CK),
            "kw": _chunk_pm(kwf[:, 128 * c:128 * (c + 1)].astype(bf), CK),
            "vw": _chunk_pm(vwf[:, 128 * c:128 * (c + 1)].astype(bf), CK),
            "ow": ow_h,
            "gatew": _chunk_pm(gatewf, CK),
            "guw": guw_h,
            "dww": dww_h,
            "cosT": cosT,
            "sinTs": sinT,
            "masks": masks,
            "rstd1": rstd1,
            "qnw": np.asarray(qn_w, dtype=np.float32)[:, None],
            "knw": np.asarray(kn_w, dtype=np.float32)[:, None],
            "expid": np.full((128, 1), float(c), dtype=np.float32),
            "prot": protm,
        })

    return in_maps


def combine(x, ys):
    x_flat = np.asarray(x, dtype=np.float32).reshape(N, C)
    ysum = np.zeros((128, CK, N), dtype=np.float32)
    for yc in ys:
        ysum += yc
    yT = ysum.transpose(1, 0, 2).reshape(C, N)
    out = x_flat + yT.T
    return out.reshape(B, T, C).astype(np.float32)


def kernel(**inputs):
    in_maps = prepare_in_maps(**inputs)
    nc = _get_nc()
    res = run_bass_kernel_spmd(nc, in_maps, core_ids=list(range(N_CORES)))
    return combine(inputs["x"], [res.results[c]["y"] for c in range(N_CORES)])
